# revision 3
# baseline (speedup 1.0000x reference)
"""Trainium2 Bass kernel for nn_Conv1d_NN_Spatial (retrieval KNN + conv1d).

Pipeline per core (data-parallel over batch, 4 batches/core):
  s[n, m] = 2*dot(x[:, n], y[:, m]) - |y[:, m]|^2      (rank key; top-8 of s
            == 8 nearest y-samples, same order/ties as reference top_k)
  via one K=65 matmul:   lhsT = [x; ones],  rhs = [2*y; -|y|^2]
  -> Max8 / FindIndex8 (DVE) per 128-row tile
  -> index reorder DMA into GPSIMD "wrapped" layout + fp16 broadcast matmul
  -> indirect_copy (GPSIMD) gathers neighbor features from x[:, indices] table
  -> 9 accumulating PE matmuls (kernel taps) + bias => out[co, n]

Self-contained: hardcodes shapes from the problem spec.
"""
import numpy as np

B, C_IN, C_OUT, N, M, K = 32, 64, 128, 4096, 1024, 9
NCORES = 8
B_LOCAL = B // NCORES
NT = N // 128  # n-tiles per batch

# dtype knobs: dot in {"float32", "float32r"}, conv in {"float32", "float16"}
DOT_DT = "float32"
CONV_DT = "float32"

_prog_cache = {}


def _build(dot_dt: str, conv_dt: str, b_local: int = B_LOCAL):
    import concourse.bacc as bacc
    import concourse.mybir as mybir
    from concourse.tile import TileContext

    fp32 = mybir.dt.float32
    fp16 = mybir.dt.float16
    u16 = mybir.dt.uint16
    cdt = fp32 if conv_dt == "float32" else fp16
    ddt = mybir.dt.float32 if dot_dt == "float32" else mybir.dt.float32r

    nc = bacc.Bacc("TRN2", target_bir_lowering=False, debug=False)

    xa_d = nc.dram_tensor("xa", [b_local, C_IN + 1, N], fp32, kind="ExternalInput").ap()
    y_d = nc.dram_tensor("y", [b_local, C_IN, M], fp32, kind="ExternalInput").ap()
    xs_d = nc.dram_tensor("xs", [b_local, C_IN, M], cdt, kind="ExternalInput").ap()
    wt_d = nc.dram_tensor("wt", [128, K * 128], cdt, kind="ExternalInput").ap()
    m2_d = nc.dram_tensor("m2", [32, 128], fp16, kind="ExternalInput").ap()
    bias_d = nc.dram_tensor("bias", [128, 1], fp32, kind="ExternalInput").ap()
    if conv_dt != "float32":
        xself_d = nc.dram_tensor("xself", [b_local, C_IN, N], cdt, kind="ExternalInput").ap()
    out_d = nc.dram_tensor("out", [b_local, C_OUT, N], fp32, kind="ExternalOutput").ap()

    n_pairs = b_local // 2

    with TileContext(nc) as tc:
        with tc.tile_pool(name="const", bufs=1) as const_pool, \
             tc.tile_pool(name="pairbuf", bufs=1) as pair_pool, \
             tc.tile_pool(name="work", bufs=3) as work_pool, \
             tc.tile_pool(name="acc", bufs=2) as acc_pool, \
             tc.tile_pool(name="pdist", bufs=2, space="PSUM") as pdist_pool, \
             tc.tile_pool(name="pconv", bufs=2, space="PSUM") as pconv_pool, \
             tc.tile_pool(name="pidx", bufs=2, space="PSUM") as pidx_pool:

            wt_s = const_pool.tile([128, K * 128], cdt, name="wt_s")
            m2_s = const_pool.tile([32, 128], fp16, name="m2_s")
            bias_s = const_pool.tile([128, 1], fp32, name="bias_s")
            ones_s = const_pool.tile([C_IN, 1], fp32, name="ones_s")
            nc.sync.dma_start(out=wt_s[:], in_=wt_d[:])
            nc.sync.dma_start(out=m2_s[:], in_=m2_d[:])
            nc.sync.dma_start(out=bias_s[:], in_=bias_d[:])
            nc.vector.memset(ones_s[:], 1.0)

            for pair in range(n_pairs):
                bb = (2 * pair, 2 * pair + 1)

                # ---- per-pair input staging -----------------------------
                xa_t, ya_t, accum_t = [], [], []
                xs_pack = pair_pool.tile([128, M], cdt, name=f"xs_pack{pair}", tag="xs_pack")
                xself_t = []
                for i, b in enumerate(bb):
                    xa_s = pair_pool.tile([C_IN + 1, N], fp32, name=f"xa{pair}_{i}", tag=f"xa{i}")
                    nc.sync.dma_start(out=xa_s[:], in_=xa_d[b])
                    xa_t.append(xa_s)
                    if conv_dt != "float32":
                        xf_s = pair_pool.tile([C_IN, N], cdt, name=f"xself{pair}_{i}", tag=f"xself{i}")
                        nc.sync.dma_start(out=xf_s[:], in_=xself_d[b])
                        xself_t.append(xf_s)
                    else:
                        xself_t.append(xa_s)

                    ya_s = pair_pool.tile([C_IN + 1, M], fp32, name=f"ya{pair}_{i}", tag=f"ya{i}")
                    nc.sync.dma_start(out=ya_s[0:C_IN, :], in_=y_d[b])
                    # rows 0..63 <- 2*y ; row 64 <- -|y|^2 = -0.25 * sum((2y)^2)
                    nc.scalar.mul(ya_s[0:C_IN, :], ya_s[0:C_IN, :], 2.0)
                    ysq = work_pool.tile([C_IN, M], fp32, name=f"ysq{pair}_{i}", tag="ysq")
                    nc.scalar.square(ysq[:], ya_s[0:C_IN, :])
                    ny_p = pdist_pool.tile([1, M], fp32, name=f"ny{pair}_{i}", tag="dist")
                    for h in range(2):
                        nc.tensor.matmul(ny_p[:, 512 * h:512 * h + 512],
                                         lhsT=ones_s[:], rhs=ysq[:, 512 * h:512 * h + 512],
                                         start=True, stop=True)
                    nc.scalar.mul(ya_s[C_IN:C_IN + 1, :], ny_p[:], -0.25)
                    ya_t.append(ya_s)

                    nc.sync.dma_start(out=xs_pack[64 * i:64 * i + 64, :], in_=xs_d[b])

                    accum_s = acc_pool.tile([C_OUT, N], fp32, name=f"accum{pair}_{i}", tag=f"accum{i}")
                    accum_t.append(accum_s)

                # ---- main loop over 32 n-tiles --------------------------
                for t in range(NT):
                    ncol = slice(128 * t, 128 * t + 128)
                    wrapped = work_pool.tile([32, 64], fp16, name=f"wr{pair}_{t}", tag="wrapped")
                    for i in range(2):
                        # distance-key matmul -> psum [128, 1024]
                        dist_p = pdist_pool.tile([128, M], fp32, name=f"dp{pair}_{t}_{i}", tag="dist")
                        lhs = xa_t[i][:, ncol]
                        rhs = ya_t[i][:]
                        if dot_dt != "float32":
                            lhs = lhs.bitcast(ddt)
                            rhs = rhs.bitcast(ddt)
                        for h in range(2):
                            nc.tensor.matmul(dist_p[:, 512 * h:512 * h + 512],
                                             lhsT=lhs, rhs=rhs[:, 512 * h:512 * h + 512],
                                             start=True, stop=True)
                        dist_s = work_pool.tile([128, M], fp32, name=f"ds{pair}_{t}_{i}", tag="dist_s")
                        nc.scalar.copy(dist_s[:], dist_p[:])

                        maxv = work_pool.tile([128, 8], fp32, name=f"mv{pair}_{t}_{i}", tag="maxv")
                        idxu = work_pool.tile([128, 8], u16, name=f"iu{pair}_{t}_{i}", tag="idxu")
                        idxh = work_pool.tile([128, 8], fp16, name=f"ih{pair}_{t}_{i}", tag="idxh")
                        nc.vector.max(maxv[:], dist_s[:])
                        nc.vector.max_index(idxu[:], maxv[:], dist_s[:])
                        nc.vector.tensor_copy(idxh[:], idxu[:])
                        # reorder [128, 8] -> wrapped [16, 64] (dumb DMA)
                        nc.sync.dma_start(out=wrapped[16 * i:16 * i + 16, :], in_=idxh[:])

                    # broadcast wrapped idx to all 16-partition groups
                    idxb_p = pidx_pool.tile([128, 64], fp32, name=f"ib{pair}_{t}", tag="idxb")
                    nc.tensor.matmul(idxb_p[:], lhsT=m2_s[:], rhs=wrapped[:], start=True, stop=True)
                    icidx = work_pool.tile([128, 64], u16, name=f"ici{pair}_{t}", tag="icidx")
                    nc.scalar.copy(icidx[:], idxb_p[:])

                    # gather: out col j = (n, k), j = 128*(n%8) + 16*k + n//8
                    gath = work_pool.tile([128, 8 * 128], cdt, name=f"g{pair}_{t}", tag="gath")
                    nc.gpsimd.indirect_copy(gath[:], xs_pack[:], icidx[:], True)

                    # conv: 9 accumulating matmuls per batch
                    for i in range(2):
                        conv_p = pconv_pool.tile([C_OUT, 128], fp32, name=f"cp{pair}_{t}_{i}", tag="conv")
                        base = 64 * i
                        # k = 0 (self): cols n = 8b + a -> AP dims (a stride 1, b stride 8)
                        self_rhs = xself_t[i][0:C_IN, ncol].rearrange("p (b a) -> p a b", a=8)
                        lhs0 = wt_s[0:64, 0:128]
                        if conv_dt == "float32" and dot_dt != "float32":
                            pass  # xa stays fp32 for conv even with f32r dot
                        nc.tensor.matmul(conv_p[:], lhsT=lhs0, rhs=self_rhs,
                                         start=True, stop=False)
                        gsl = gath[base:base + 64, :].rearrange("p (a b) -> p a b", a=8)
                        for k in range(1, K):
                            nc.tensor.matmul(conv_p[:],
                                             lhsT=wt_s[base:base + 64, 128 * k:128 * k + 128],
                                             rhs=gsl[:, :, 16 * (k - 1):16 * (k - 1) + 16],
                                             start=False, stop=(k == K - 1))
                        # unpermute psum cols (c = 16a + b  ->  n = 8b + a) + bias
                        src = conv_p[:].rearrange("p (a b) -> p a b", a=8)
                        dst = accum_t[i][:, ncol].rearrange("p (b a) -> p a b", a=8)
                        import concourse.mybir as _mb
                        nc.scalar.activation(dst, src, _mb.ActivationFunctionType.Identity,
                                             bias=bias_s[:], scale=1.0)

                for i, b in enumerate(bb):
                    nc.sync.dma_start(out=out_d[b], in_=accum_t[i][:])

    nc.compile()
    return nc


def _host_prep(x, y, indices, W, b, conv_dt):
    x = np.ascontiguousarray(np.asarray(x, dtype=np.float32))
    y = np.ascontiguousarray(np.asarray(y, dtype=np.float32))
    W = np.asarray(W, dtype=np.float32)
    b = np.asarray(b, dtype=np.float32)
    idx = np.asarray(indices).astype(np.int64)

    cnp = np.float32 if conv_dt == "float32" else np.float16
    xa = np.concatenate([x, np.ones((B, 1, N), np.float32)], axis=1)  # [B, 65, N]
    xs = np.ascontiguousarray(x[:, :, idx]).astype(cnp)               # [B, 64, M]
    # wt[ci(+64 dup), 128*k + co] = W[co, ci, k]
    wt_half = np.transpose(W, (2, 1, 0)).reshape(K * C_IN, C_OUT)     # [(k ci), co]
    wt = np.zeros((128, K * 128), cnp)
    for k in range(K):
        blk = wt_half[k * C_IN:(k + 1) * C_IN, :]                     # [ci, co]
        wt[0:64, 128 * k:128 * k + 128] = blk
        wt[64:128, 128 * k:128 * k + 128] = blk
    m2 = np.zeros((32, 128), np.float16)
    for p in range(128):
        m2[(p // 64) * 16 + p % 16, p] = 1.0
    bias = b.reshape(C_OUT, 1)
    xself = x.astype(cnp) if conv_dt != "float32" else None
    return xa, xs, wt, m2, bias, xself


def kernel(x, y, indices, W, b):
    from concourse import bass_utils

    key = (DOT_DT, CONV_DT)
    if key not in _prog_cache:
        _prog_cache[key] = _build(*key)
    nc = _prog_cache[key]

    xa, xs, wt, m2, bias, xself = _host_prep(x, y, indices, W, b, CONV_DT)
    y = np.ascontiguousarray(np.asarray(y, dtype=np.float32))

    in_maps = []
    for c in range(NCORES):
        sl = slice(c * B_LOCAL, (c + 1) * B_LOCAL)
        m = {"xa": np.ascontiguousarray(xa[sl]),
             "y": y[sl],
             "xs": np.ascontiguousarray(xs[sl]),
             "wt": wt, "m2": m2, "bias": bias}
        if xself is not None:
            m["xself"] = np.ascontiguousarray(xself[sl])
        in_maps.append(m)

    res = bass_utils.run_bass_kernel_spmd(nc, in_maps, list(range(NCORES)))
    out = np.concatenate([r["out"] for r in res.results], axis=0)
    return out
